# revision 19
# baseline (speedup 1.0000x reference)
"""VQ-codebook kernel for Trainium2 (Bass/Tile), 8 NeuronCores data-parallel.

Computes, for x:[512,365,10], mask:[512,365], prototypes:[64,365,10]:
    d[b,p]  = sum_{t,c} mask[b,t] * (x[b,t,c] - proto[p,t,c])^2
    idx[b]  = argmin_p d[b,p]
    out_seq = prototypes[idx]
and returns the reference's 6-tuple (out_seq, x, d, idx, label, mask).

Strategy (per core, B sharded 512 -> 8x64):
  d = x2[b] - 2*xp[b,p] + p2[b,p], all via fp32 PE matmuls with the
  contraction dim (t*c = 3650, zero-padded to 3712 = 29*128) on partitions:
    - chain1: lhsT = (mask*x)^T chunks, rhs = [-2*proto^T | x^T] -> S[b, 0:64]
      accumulates -2*xp, S[b, 64:128] accumulates the Gram matrix G whose
      diagonal is x2.
    - chain2: lhsT = mask10^T chunks, rhs = (proto^T)^2 -> accumulates p2
      into S[b, 0:64].
  x2 = diag(G) via tensor_tensor_reduce with the identity; d = S[:,0:64]+x2.
  argmin via vector.max/max_index on -d; gather via exact one-hot fp32
  matmul: onehot^T[p,b] = (argmin[b] == p), out_seq = onehot^T.T @ proto.
  (indirect-DMA gather crashes the device on this runtime path: probe.py
  reproduced NRT_EXEC_UNIT_UNRECOVERABLE with indirect_dma_start alone.)

Transposed operands are packed on the host (pure layout, no arithmetic):
DRAM [128, 29, n] so each SBUF partition loads linearly.
"""

import os
import sys

import numpy as np

for _p in ("/opt/trn_rl_repo",):
    if os.path.isdir(_p) and _p not in sys.path:
        sys.path.insert(0, _p)

B, T, C, P = 512, 365, 10, 64
NCORES = 8
BL = B // NCORES  # 64 samples per core
TC = T * C  # 3650
KCH = 29  # contraction chunks of 128
KPAD = KCH * 128  # 3712

_PROG = None


def _build_program():
    import concourse.bacc as bacc
    import concourse.mybir as mybir
    import concourse.tile as tile
    from concourse import bass

    f32 = mybir.dt.float32
    nc = bacc.Bacc("TRN2", target_bir_lowering=False, debug=False,
                   num_devices=NCORES)

    xt_d = nc.dram_tensor("xt", [128, KCH, BL], f32, kind="ExternalInput")
    mt_d = nc.dram_tensor("mt10", [128, KCH, BL], f32, kind="ExternalInput")
    pt_d = nc.dram_tensor("protot", [128, KCH, P], f32, kind="ExternalInput")
    pr_d = nc.dram_tensor("proto", [P, TC], f32, kind="ExternalInput")
    id_d = nc.dram_tensor("ident", [BL, BL], f32, kind="ExternalInput")
    oseq_d = nc.dram_tensor("outseq", [BL, TC], f32, kind="ExternalOutput")
    dist_d = nc.dram_tensor("dist", [BL, P], f32, kind="ExternalOutput")
    idx_d = nc.dram_tensor("idx", [BL], mybir.dt.int32, kind="ExternalOutput")

    NW = P + BL  # 128 columns per chain1 rhs chunk

    GROUPS = [(0, 8), (8, 16), (16, 24), (24, KCH)]

    with tile.TileContext(nc) as tc:
        with tc.tile_pool(name="sb", bufs=1) as sb, \
             tc.tile_pool(name="sbg", bufs=3) as sbg, \
             tc.tile_pool(name="sbm", bufs=4) as sbm, \
             tc.tile_pool(name="sbo", bufs=4) as sbo, \
             tc.tile_pool(name="ps", bufs=1, space="PSUM") as ps, \
             tc.tile_pool(name="ps2", bufs=4, space="PSUM") as ps2:
            ident = sb.tile([BL, BL], f32)
            pr_sb = sb.tile([P, TC], f32)
            nc.sync.dma_start(out=ident[:], in_=id_d[:, :])
            nc.sync.dma_start(out=pr_sb[:], in_=pr_d[:, :])

            # per-group loads + prep, pipelined via pool bufs
            groups = []
            for (k0, k1) in GROUPS:
                kn = k1 - k0
                rhs_g = sbg.tile([128, kn * NW], f32, tag="rhs")
                m_g = sbm.tile([128, kn * BL], f32, tag="m")
                pt_g = sbg.tile([128, kn * P], f32, tag="pt")
                sq_g = sbm.tile([128, kn * P], f32, tag="sq")
                mx_g = sbg.tile([128, kn * BL], f32, tag="mx")
                rhs3 = rhs_g[:].rearrange("r (k n) -> r k n", n=NW)
                nc.sync.dma_start(out=rhs3[:, :, P:NW],
                                  in_=xt_d[:, k0:k1, :])
                nc.sync.dma_start(
                    out=m_g[:].rearrange("r (k n) -> r k n", n=BL),
                    in_=mt_d[:, k0:k1, :])
                nc.sync.dma_start(
                    out=pt_g[:].rearrange("r (k n) -> r k n", n=P),
                    in_=pt_d[:, k0:k1, :])
                nc.scalar.mul(
                    out=rhs3[:, :, 0:P],
                    in_=pt_g[:].rearrange("r (k n) -> r k n", n=P),
                    mul=-2.0)
                nc.scalar.square(out=sq_g[:], in_=pt_g[:])
                nc.vector.tensor_tensor(
                    out=mx_g[:].rearrange("r (k n) -> r k n", n=BL),
                    in0=rhs3[:, :, P:NW],
                    in1=m_g[:].rearrange("r (k n) -> r k n", n=BL),
                    op=mybir.AluOpType.mult)
                groups.append((kn, rhs_g, m_g, sq_g, mx_g))

            # chain1: S = [-2*xp | G], one clean accumulation group
            S = ps.tile([BL, NW], f32)
            first = True
            for gi, (kn, rhs_g, m_g, sq_g, mx_g) in enumerate(groups):
                for j in range(kn):
                    nc.tensor.matmul(
                        out=S[:, :],
                        lhsT=mx_g[:, j * BL:(j + 1) * BL],
                        rhs=rhs_g[:, j * NW:(j + 1) * NW],
                        start=first,
                        stop=(gi == len(groups) - 1 and j == kn - 1))
                    first = False
            # chain2: S2 = p2 = mask10 @ protosq^T, its own group
            S2 = ps.tile([BL, P], f32)
            first = True
            for gi, (kn, rhs_g, m_g, sq_g, mx_g) in enumerate(groups):
                for j in range(kn):
                    nc.tensor.matmul(
                        out=S2[:, :],
                        lhsT=m_g[:, j * BL:(j + 1) * BL],
                        rhs=sq_g[:, j * P:(j + 1) * P],
                        start=first,
                        stop=(gi == len(groups) - 1 and j == kn - 1))
                    first = False

            # evict PSUM via plain copies only (ttr/tensor_scalar reading
            # PSUM crash the device on this runtime path — see bisect2.py)
            s_sb = sb.tile([BL, NW], f32)
            s2_sb = sb.tile([BL, P], f32)
            nc.vector.tensor_copy(out=s_sb[:], in_=S[:, :])
            nc.scalar.copy(out=s2_sb[:], in_=S2[:, :])

            # x2 = diag(G) = rowsum(G * I)
            gscratch = sb.tile([BL, BL], f32)
            x2col = sb.tile([BL, 1], f32)
            nc.vector.tensor_tensor(
                out=gscratch[:], in0=s_sb[:, P:NW], in1=ident[:],
                op=mybir.AluOpType.mult)
            nc.vector.tensor_reduce(
                out=x2col[:], in_=gscratch[:],
                axis=mybir.AxisListType.X, op=mybir.AluOpType.add)

            # d = (-2xp + x2) + p2 ; negd = -d
            d_sb = sb.tile([BL, P], f32)
            negd = sb.tile([BL, P], f32)
            nc.vector.scalar_tensor_tensor(
                out=d_sb[:], in0=s_sb[:, 0:P],
                scalar=x2col[:, 0:1], in1=s2_sb[:],
                op0=mybir.AluOpType.add, op1=mybir.AluOpType.add)
            nc.vector.tensor_scalar(
                out=negd[:], in0=d_sb[:],
                scalar1=-1.0, scalar2=None,
                op0=mybir.AluOpType.mult)

            # argmin d = first index of max(-d)  (for the idx output)
            vmax = sb.tile([BL, 8], f32)
            idx8 = sb.tile([BL, 8], mybir.dt.uint32)
            idxi = sb.tile([BL, 1], mybir.dt.int32)
            nc.vector.max(vmax[:], negd[:])
            nc.vector.max_index(idx8[:], vmax[:], negd[:])
            nc.vector.tensor_copy(out=idxi[:], in_=idx8[:, 0:1])

            # one-hot[b, p] = (-d[b,p] == max_b(-d)) — exactly one hit per
            # row (min distance gaps >> fp32 error; verified vs reference),
            # then one PE transpose to get onehot^T for the gather matmul.
            onehot_n = sb.tile([BL, P], f32)
            nc.vector.tensor_scalar(
                out=onehot_n[:], in0=negd[:],
                scalar1=vmax[:, 0:1], scalar2=None,
                op0=mybir.AluOpType.is_equal)
            ps_oh = ps.tile([P, BL], f32)
            nc.tensor.transpose(out=ps_oh[:], in_=onehot_n[:],
                                identity=ident[:])
            onehot_t = sb.tile([P, BL], f32)
            nc.vector.tensor_copy(out=onehot_t[:], in_=ps_oh[:])

            # out_seq = onehot^T.T @ proto  (exact: one 1.0 per column),
            # streamed out chunk by chunk
            NGC = 512
            for j in range((TC + NGC - 1) // NGC):
                n0 = j * NGC
                n1 = min(n0 + NGC, TC)
                ps_g = ps2.tile([BL, NGC], f32, tag="ps_g")
                nc.tensor.matmul(out=ps_g[:, 0:n1 - n0], lhsT=onehot_t[:],
                                 rhs=pr_sb[:, n0:n1], start=True, stop=True)
                g_sb = sbo.tile([BL, NGC], f32, tag="g_sb")
                if j % 2 == 0:
                    nc.scalar.copy(out=g_sb[:, 0:n1 - n0],
                                   in_=ps_g[:, 0:n1 - n0])
                else:
                    nc.vector.tensor_copy(out=g_sb[:, 0:n1 - n0],
                                          in_=ps_g[:, 0:n1 - n0])
                nc.sync.dma_start(out=oseq_d[:, n0:n1],
                                  in_=g_sb[:, 0:n1 - n0])

            nc.sync.dma_start(out=dist_d[:, :], in_=d_sb[:])
            nc.sync.dma_start(out=idx_d[:, None], in_=idxi[:])

    nc.compile()
    return nc


def _get_program():
    global _PROG
    if _PROG is None:
        _PROG = _build_program()
    return _PROG


def _pack_T(a):
    """[n, TC] row-major -> transposed, zero-padded, chunk-packed [128, KCH, n]."""
    at = np.zeros((KPAD, a.shape[0]), np.float32)
    at[:TC] = a.T
    return np.ascontiguousarray(at.reshape(KCH, 128, -1).transpose(1, 0, 2))


# set by test harness: trace the next run and stash BassKernelResults here
TRACE = False
LAST_RESULTS = None


def kernel(**inputs):
    global LAST_RESULTS
    from concourse.bass_utils import run_bass_kernel_spmd

    x = np.ascontiguousarray(np.asarray(inputs["input_seq"], dtype=np.float32))
    mask = np.ascontiguousarray(np.asarray(inputs["mask"], dtype=np.float32))
    label = np.asarray(inputs["label"])
    proto = np.ascontiguousarray(
        np.asarray(inputs["prototypes"], dtype=np.float32))

    nc = _get_program()
    pr2 = np.ascontiguousarray(proto.reshape(P, TC))
    protot_packed = _pack_T(pr2)
    ident = np.eye(BL, dtype=np.float32)

    in_maps = []
    for c in range(NCORES):
        xc = x[c * BL:(c + 1) * BL].reshape(BL, TC)
        mc = np.repeat(mask[c * BL:(c + 1) * BL], C, axis=1)
        in_maps.append({
            "xt": _pack_T(xc),
            "mt10": _pack_T(mc),
            "protot": protot_packed,
            "proto": pr2,
            "ident": ident,
        })

    res = run_bass_kernel_spmd(nc, in_maps, list(range(NCORES)), trace=TRACE)
    LAST_RESULTS = res

    outseq = np.concatenate(
        [res.results[c]["outseq"] for c in range(NCORES)], axis=0
    ).reshape(B, T, C)
    dist = np.concatenate(
        [res.results[c]["dist"] for c in range(NCORES)], axis=0)
    idx = np.concatenate(
        [res.results[c]["idx"] for c in range(NCORES)], axis=0).astype(np.int32)

    return (outseq, x, dist, idx, label, mask)


# revision 22
# speedup vs baseline: 1.0453x; 1.0453x over previous
"""VQ-codebook kernel for Trainium2 (Bass/Tile), 8 NeuronCores data-parallel.

Computes, for x:[512,365,10], mask:[512,365], prototypes:[64,365,10]:
    d[b,p]  = sum_{t,c} mask[b,t] * (x[b,t,c] - proto[p,t,c])^2
    idx[b]  = argmin_p d[b,p]
    out_seq = prototypes[idx]
and returns the reference's 6-tuple (out_seq, x, d, idx, label, mask).

Strategy (per core, B sharded 512 -> 8x64):
  d = x2[b] - 2*xp[b,p] + p2[b,p], all via fp32 PE matmuls with the
  contraction dim (t*c = 3650, zero-padded to 3712 = 29*128) on partitions:
    - chain1: lhsT = (mask*x)^T chunks, rhs = [-2*proto^T | x^T] -> S[b, 0:64]
      accumulates -2*xp, S[b, 64:128] accumulates the Gram matrix G whose
      diagonal is x2.
    - chain2: lhsT = mask10^T chunks, rhs = (proto^T)^2 -> accumulates p2
      into S[b, 0:64].
  x2 = diag(G) = rowsum(G * I); d = -2xp + x2 + p2; argmin via
  vector.max/max_index on -d; gather via exact one-hot fp32 matmul:
  onehot[b,p] = (-d[b,p] == max(-d[b,:])), PE-transposed, then
  out_seq = onehot^T.T @ proto (exact: products are v*1.0 summed with 0s).
  (indirect-DMA gather crashes the device on this runtime path: probe.py
  reproduced NRT_EXEC_UNIT_UNRECOVERABLE with indirect_dma_start alone.)

Transposed operands are packed on the host (pure layout, no arithmetic):
DRAM [128, 29, n] so each SBUF partition loads linearly.
"""

import os
import sys

import numpy as np

for _p in ("/opt/trn_rl_repo",):
    if os.path.isdir(_p) and _p not in sys.path:
        sys.path.insert(0, _p)

B, T, C, P = 512, 365, 10, 64
NCORES = 8
BL = B // NCORES  # 64 samples per core
TC = T * C  # 3650
KCH = 29  # contraction chunks of 128
KPAD = KCH * 128  # 3712

_PROG = None


def _build_program():
    import concourse.bacc as bacc
    import concourse.mybir as mybir
    import concourse.tile as tile
    from concourse import bass

    f32 = mybir.dt.float32
    nc = bacc.Bacc("TRN2", target_bir_lowering=False, debug=False,
                   num_devices=NCORES)

    xt_d = nc.dram_tensor("xt", [128, KCH, BL], f32, kind="ExternalInput")
    mt_d = nc.dram_tensor("mt10", [128, KCH, BL], f32, kind="ExternalInput")
    pt_d = nc.dram_tensor("protot", [128, KCH, P], f32, kind="ExternalInput")
    pr_d = nc.dram_tensor("proto", [P, TC], f32, kind="ExternalInput")
    id_d = nc.dram_tensor("ident", [BL, BL], f32, kind="ExternalInput")
    oseq_d = nc.dram_tensor("outseq", [BL, TC], f32, kind="ExternalOutput")
    dist_d = nc.dram_tensor("dist", [BL, P], f32, kind="ExternalOutput")
    idx_d = nc.dram_tensor("idx", [BL], mybir.dt.int32, kind="ExternalOutput")

    NW = P + BL  # 128 columns per chain1 rhs chunk

    GROUPS = [(0, 8), (8, 16), (16, 24), (24, KCH)]

    with tile.TileContext(nc) as tc:
        with tc.tile_pool(name="sb", bufs=1) as sb, \
             tc.tile_pool(name="sbg", bufs=3) as sbg, \
             tc.tile_pool(name="sbm", bufs=4) as sbm, \
             tc.tile_pool(name="sbo", bufs=4) as sbo, \
             tc.tile_pool(name="ps", bufs=1, space="PSUM") as ps, \
             tc.tile_pool(name="ps2", bufs=4, space="PSUM") as ps2:
            ident = sb.tile([BL, BL], f32)
            pr_sb = sb.tile([P, TC], f32)
            # per-group loads + prep, pipelined via pool bufs
            # (ident/proto loads issued after these: they are only needed
            # by the late transpose/gather, so keep front DMA BW for x/mask)
            groups = []
            for (k0, k1) in GROUPS:
                kn = k1 - k0
                rhs_g = sbg.tile([128, kn * NW], f32, tag="rhs")
                m_g = sbm.tile([128, kn * BL], f32, tag="m")
                pt_g = sbg.tile([128, kn * P], f32, tag="pt")
                sq_g = sbm.tile([128, kn * P], f32, tag="sq")
                mx_g = sbg.tile([128, kn * BL], f32, tag="mx")
                rhs3 = rhs_g[:].rearrange("r (k n) -> r k n", n=NW)
                nc.sync.dma_start(out=rhs3[:, :, P:NW],
                                  in_=xt_d[:, k0:k1, :])
                nc.sync.dma_start(
                    out=m_g[:].rearrange("r (k n) -> r k n", n=BL),
                    in_=mt_d[:, k0:k1, :])
                nc.sync.dma_start(
                    out=pt_g[:].rearrange("r (k n) -> r k n", n=P),
                    in_=pt_d[:, k0:k1, :])
                nc.scalar.mul(
                    out=rhs3[:, :, 0:P],
                    in_=pt_g[:].rearrange("r (k n) -> r k n", n=P),
                    mul=-2.0)
                nc.scalar.square(out=sq_g[:], in_=pt_g[:])
                nc.vector.tensor_tensor(
                    out=mx_g[:].rearrange("r (k n) -> r k n", n=BL),
                    in0=rhs3[:, :, P:NW],
                    in1=m_g[:].rearrange("r (k n) -> r k n", n=BL),
                    op=mybir.AluOpType.mult)
                groups.append((kn, rhs_g, m_g, sq_g, mx_g))

            nc.sync.dma_start(out=ident[:], in_=id_d[:, :])
            nc.sync.dma_start(out=pr_sb[:], in_=pr_d[:, :])

            # chain1: S = [-2*xp | G], one clean accumulation group
            S = ps.tile([BL, NW], f32)
            first = True
            for gi, (kn, rhs_g, m_g, sq_g, mx_g) in enumerate(groups):
                for j in range(kn):
                    nc.tensor.matmul(
                        out=S[:, :],
                        lhsT=mx_g[:, j * BL:(j + 1) * BL],
                        rhs=rhs_g[:, j * NW:(j + 1) * NW],
                        start=first,
                        stop=(gi == len(groups) - 1 and j == kn - 1))
                    first = False
            # chain2: S2 = p2 = mask10 @ protosq^T, its own group
            S2 = ps.tile([BL, P], f32)
            first = True
            for gi, (kn, rhs_g, m_g, sq_g, mx_g) in enumerate(groups):
                for j in range(kn):
                    nc.tensor.matmul(
                        out=S2[:, :],
                        lhsT=m_g[:, j * BL:(j + 1) * BL],
                        rhs=sq_g[:, j * P:(j + 1) * P],
                        start=first,
                        stop=(gi == len(groups) - 1 and j == kn - 1))
                    first = False

            # evict PSUM via plain copies only (tensor_tensor_reduce with a
            # PSUM operand crashes the device on this path — see bisect2.py)
            s_sb = sb.tile([BL, NW], f32)
            s2_sb = sb.tile([BL, P], f32)
            nc.vector.tensor_copy(out=s_sb[:], in_=S[:, :])
            nc.scalar.copy(out=s2_sb[:], in_=S2[:, :])

            # x2 = diag(G) = rowsum(G * I)
            gscratch = sb.tile([BL, BL], f32)
            x2col = sb.tile([BL, 1], f32)
            nc.vector.tensor_tensor(
                out=gscratch[:], in0=s_sb[:, P:NW], in1=ident[:],
                op=mybir.AluOpType.mult)
            nc.vector.tensor_reduce(
                out=x2col[:], in_=gscratch[:],
                axis=mybir.AxisListType.X, op=mybir.AluOpType.add)

            # d = (-2xp + x2) + p2 ; negd = -d
            d_sb = sb.tile([BL, P], f32)
            negd = sb.tile([BL, P], f32)
            nc.vector.scalar_tensor_tensor(
                out=d_sb[:], in0=s_sb[:, 0:P],
                scalar=x2col[:, 0:1], in1=s2_sb[:],
                op0=mybir.AluOpType.add, op1=mybir.AluOpType.add)
            nc.vector.tensor_scalar(
                out=negd[:], in0=d_sb[:],
                scalar1=-1.0, scalar2=None,
                op0=mybir.AluOpType.mult)

            # argmin d = first index of max(-d)  (for the idx output)
            vmax = sb.tile([BL, 8], f32)
            idx8 = sb.tile([BL, 8], mybir.dt.uint32)
            idxi = sb.tile([BL, 1], mybir.dt.int32)
            nc.vector.max(vmax[:], negd[:])
            nc.vector.max_index(idx8[:], vmax[:], negd[:])
            nc.vector.tensor_copy(out=idxi[:], in_=idx8[:, 0:1])

            # one-hot[b, p] = (-d[b,p] == max_b(-d)) — exactly one hit per
            # row (min distance gaps >> fp32 error; verified vs reference),
            # then one PE transpose to get onehot^T for the gather matmul.
            onehot_n = sb.tile([BL, P], f32)
            nc.vector.tensor_scalar(
                out=onehot_n[:], in0=negd[:],
                scalar1=vmax[:, 0:1], scalar2=None,
                op0=mybir.AluOpType.is_equal)
            ps_oh = ps.tile([P, BL], f32)
            nc.tensor.transpose(out=ps_oh[:], in_=onehot_n[:],
                                identity=ident[:])
            onehot_t = sb.tile([P, BL], f32)
            nc.vector.tensor_copy(out=onehot_t[:], in_=ps_oh[:])

            # out_seq = onehot^T.T @ proto  (exact: one 1.0 per column),
            # streamed out chunk by chunk
            NGC = 512
            for j in range((TC + NGC - 1) // NGC):
                n0 = j * NGC
                n1 = min(n0 + NGC, TC)
                ps_g = ps2.tile([BL, NGC], f32, tag="ps_g")
                nc.tensor.matmul(out=ps_g[:, 0:n1 - n0], lhsT=onehot_t[:],
                                 rhs=pr_sb[:, n0:n1], start=True, stop=True)
                g_sb = sbo.tile([BL, NGC], f32, tag="g_sb")
                if j % 2 == 0:
                    nc.scalar.copy(out=g_sb[:, 0:n1 - n0],
                                   in_=ps_g[:, 0:n1 - n0])
                else:
                    nc.vector.tensor_copy(out=g_sb[:, 0:n1 - n0],
                                          in_=ps_g[:, 0:n1 - n0])
                nc.sync.dma_start(out=oseq_d[:, n0:n1],
                                  in_=g_sb[:, 0:n1 - n0])

            nc.sync.dma_start(out=dist_d[:, :], in_=d_sb[:])
            nc.sync.dma_start(out=idx_d[:, None], in_=idxi[:])

    nc.compile()
    return nc


def _get_program():
    global _PROG
    if _PROG is None:
        _PROG = _build_program()
    return _PROG


def _pack_T(a):
    """[n, TC] row-major -> transposed, zero-padded, chunk-packed [128, KCH, n]."""
    at = np.zeros((KPAD, a.shape[0]), np.float32)
    at[:TC] = a.T
    return np.ascontiguousarray(at.reshape(KCH, 128, -1).transpose(1, 0, 2))


# set by test harness: trace the next run and stash BassKernelResults here
TRACE = False
LAST_RESULTS = None


def kernel(**inputs):
    global LAST_RESULTS
    from concourse.bass_utils import run_bass_kernel_spmd

    x = np.ascontiguousarray(np.asarray(inputs["input_seq"], dtype=np.float32))
    mask = np.ascontiguousarray(np.asarray(inputs["mask"], dtype=np.float32))
    label = np.asarray(inputs["label"])
    proto = np.ascontiguousarray(
        np.asarray(inputs["prototypes"], dtype=np.float32))

    nc = _get_program()
    pr2 = np.ascontiguousarray(proto.reshape(P, TC))
    protot_packed = _pack_T(pr2)
    ident = np.eye(BL, dtype=np.float32)

    in_maps = []
    for c in range(NCORES):
        xc = x[c * BL:(c + 1) * BL].reshape(BL, TC)
        mc = np.repeat(mask[c * BL:(c + 1) * BL], C, axis=1)
        in_maps.append({
            "xt": _pack_T(xc),
            "mt10": _pack_T(mc),
            "protot": protot_packed,
            "proto": pr2,
            "ident": ident,
        })

    res = run_bass_kernel_spmd(nc, in_maps, list(range(NCORES)), trace=TRACE)
    LAST_RESULTS = res

    outseq = np.concatenate(
        [res.results[c]["outseq"] for c in range(NCORES)], axis=0
    ).reshape(B, T, C)
    dist = np.concatenate(
        [res.results[c]["dist"] for c in range(NCORES)], axis=0)
    idx = np.concatenate(
        [res.results[c]["idx"] for c in range(NCORES)], axis=0).astype(np.int32)

    return (outseq, x, dist, idx, label, mask)


# revision 24
# speedup vs baseline: 1.1766x; 1.1257x over previous
"""VQ-codebook kernel for Trainium2 (Bass/Tile), 8 NeuronCores data-parallel.

Computes, for x:[512,365,10], mask:[512,365], prototypes:[64,365,10]:
    d[b,p]  = sum_{t,c} mask[b,t] * (x[b,t,c] - proto[p,t,c])^2
    idx[b]  = argmin_p d[b,p]
    out_seq = prototypes[idx]
and returns the reference's 6-tuple (out_seq, x, d, idx, label, mask).

Strategy (per core, B sharded 512 -> 8x64):
  d = x2[b] - 2*xp[b,p] + p2[b,p], all via fp32 PE matmuls with the
  contraction dim (t*c = 3650, zero-padded to 3712 = 29*128) on partitions:
    - chain1: lhsT = (mask*x)^T chunks, rhs = [-2*proto^T | x^T] -> S[b, 0:64]
      accumulates -2*xp, S[b, 64:128] accumulates the Gram matrix G whose
      diagonal is x2.
    - chain2: lhsT = mask10^T chunks, rhs = (proto^T)^2 -> accumulates p2
      into S[b, 0:64].
  x2 = diag(G) = rowsum(G * I); d = -2xp + x2 + p2; argmin via
  vector.max/max_index on -d; gather via exact one-hot fp32 matmul:
  onehot[b,p] = (-d[b,p] == max(-d[b,:])), PE-transposed, then
  out_seq = onehot^T.T @ proto (exact: products are v*1.0 summed with 0s).
  (indirect-DMA gather crashes the device on this runtime path: probe.py
  reproduced NRT_EXEC_UNIT_UNRECOVERABLE with indirect_dma_start alone.)

Transposed operands are packed on the host (pure layout, no arithmetic):
DRAM [128, 29, n] so each SBUF partition loads linearly.
"""

import os
import sys

import numpy as np

for _p in ("/opt/trn_rl_repo",):
    if os.path.isdir(_p) and _p not in sys.path:
        sys.path.insert(0, _p)

B, T, C, P = 512, 365, 10, 64
NCORES = 8
BL = B // NCORES  # 64 samples per core
TC = T * C  # 3650
KCH = 29  # contraction chunks of 128
KPAD = KCH * 128  # 3712

_PROG = None


def _build_program():
    import concourse.bacc as bacc
    import concourse.mybir as mybir
    import concourse.tile as tile
    from concourse import bass

    f32 = mybir.dt.float32
    nc = bacc.Bacc("TRN2", target_bir_lowering=False, debug=False,
                   num_devices=NCORES)

    xt_d = nc.dram_tensor("xt", [128, KCH, BL], f32, kind="ExternalInput")
    mt_d = nc.dram_tensor("mt10", [128, KCH, BL], f32, kind="ExternalInput")
    pt_d = nc.dram_tensor("protot", [128, KCH, P], f32, kind="ExternalInput")
    pr_d = nc.dram_tensor("proto", [P, TC], f32, kind="ExternalInput")
    id_d = nc.dram_tensor("ident", [BL, BL], f32, kind="ExternalInput")
    oseq_d = nc.dram_tensor("outseq", [BL, TC], f32, kind="ExternalOutput")
    dist_d = nc.dram_tensor("dist", [BL, P], f32, kind="ExternalOutput")
    idx_d = nc.dram_tensor("idx", [BL], mybir.dt.int32, kind="ExternalOutput")

    NW = P + BL  # 128 columns per chain1 rhs chunk

    GROUPS = [(0, 8), (8, 16), (16, 24), (24, KCH)]

    with tile.TileContext(nc) as tc:
        with tc.tile_pool(name="sb", bufs=1) as sb, \
             tc.tile_pool(name="sbg", bufs=3) as sbg, \
             tc.tile_pool(name="sbm", bufs=4) as sbm, \
             tc.tile_pool(name="sbo", bufs=4) as sbo, \
             tc.tile_pool(name="ps", bufs=1, space="PSUM") as ps, \
             tc.tile_pool(name="ps2", bufs=4, space="PSUM") as ps2:
            ident = sb.tile([BL, BL], f32)
            pr_sb = sb.tile([P, TC], f32)
            # PE HAM warmup: keep TensorE active during the DMA front so
            # the clock gate is released before the real chains issue
            wm = sb.tile([BL, BL], f32)
            nc.vector.memset(wm[:], 0.0)
            ps_w = ps.tile([BL, BL], f32)
            for _ in range(40):
                nc.tensor.matmul(out=ps_w[:], lhsT=wm[:], rhs=wm[:],
                                 start=True, stop=True)

            # per-group loads + prep, pipelined via pool bufs
            # (ident/proto loads issued after these: they are only needed
            # by the late transpose/gather, so keep front DMA BW for x/mask)
            groups = []
            for (k0, k1) in GROUPS:
                kn = k1 - k0
                rhs_g = sbg.tile([128, kn * NW], f32, tag="rhs")
                m_g = sbm.tile([128, kn * BL], f32, tag="m")
                pt_g = sbg.tile([128, kn * P], f32, tag="pt")
                sq_g = sbm.tile([128, kn * P], f32, tag="sq")
                mx_g = sbg.tile([128, kn * BL], f32, tag="mx")
                rhs3 = rhs_g[:].rearrange("r (k n) -> r k n", n=NW)
                nc.sync.dma_start(out=rhs3[:, :, P:NW],
                                  in_=xt_d[:, k0:k1, :])
                nc.sync.dma_start(
                    out=m_g[:].rearrange("r (k n) -> r k n", n=BL),
                    in_=mt_d[:, k0:k1, :])
                nc.sync.dma_start(
                    out=pt_g[:].rearrange("r (k n) -> r k n", n=P),
                    in_=pt_d[:, k0:k1, :])
                nc.scalar.mul(
                    out=rhs3[:, :, 0:P],
                    in_=pt_g[:].rearrange("r (k n) -> r k n", n=P),
                    mul=-2.0)
                nc.scalar.square(out=sq_g[:], in_=pt_g[:])
                nc.vector.tensor_tensor(
                    out=mx_g[:].rearrange("r (k n) -> r k n", n=BL),
                    in0=rhs3[:, :, P:NW],
                    in1=m_g[:].rearrange("r (k n) -> r k n", n=BL),
                    op=mybir.AluOpType.mult)
                groups.append((kn, rhs_g, m_g, sq_g, mx_g))

            nc.sync.dma_start(out=ident[:], in_=id_d[:, :])
            nc.sync.dma_start(out=pr_sb[:], in_=pr_d[:, :])

            # chain1: S = [-2*xp | G], one clean accumulation group
            S = ps.tile([BL, NW], f32)
            first = True
            for gi, (kn, rhs_g, m_g, sq_g, mx_g) in enumerate(groups):
                for j in range(kn):
                    nc.tensor.matmul(
                        out=S[:, :],
                        lhsT=mx_g[:, j * BL:(j + 1) * BL],
                        rhs=rhs_g[:, j * NW:(j + 1) * NW],
                        start=first,
                        stop=(gi == len(groups) - 1 and j == kn - 1))
                    first = False
            # chain2: S2 = p2 = mask10 @ protosq^T, its own group
            S2 = ps.tile([BL, P], f32)
            first = True
            for gi, (kn, rhs_g, m_g, sq_g, mx_g) in enumerate(groups):
                for j in range(kn):
                    nc.tensor.matmul(
                        out=S2[:, :],
                        lhsT=m_g[:, j * BL:(j + 1) * BL],
                        rhs=sq_g[:, j * P:(j + 1) * P],
                        start=first,
                        stop=(gi == len(groups) - 1 and j == kn - 1))
                    first = False

            # evict PSUM via plain copies only (tensor_tensor_reduce with a
            # PSUM operand crashes the device on this path — see bisect2.py)
            s_sb = sb.tile([BL, NW], f32)
            s2_sb = sb.tile([BL, P], f32)
            nc.vector.tensor_copy(out=s_sb[:], in_=S[:, :])
            nc.scalar.copy(out=s2_sb[:], in_=S2[:, :])

            # x2 = diag(G) = rowsum(G * I)
            gscratch = sb.tile([BL, BL], f32)
            x2col = sb.tile([BL, 1], f32)
            nc.vector.tensor_tensor(
                out=gscratch[:], in0=s_sb[:, P:NW], in1=ident[:],
                op=mybir.AluOpType.mult)
            nc.vector.tensor_reduce(
                out=x2col[:], in_=gscratch[:],
                axis=mybir.AxisListType.X, op=mybir.AluOpType.add)

            # d = (-2xp + x2) + p2 ; negd = -d
            d_sb = sb.tile([BL, P], f32)
            negd = sb.tile([BL, P], f32)
            nc.vector.scalar_tensor_tensor(
                out=d_sb[:], in0=s_sb[:, 0:P],
                scalar=x2col[:, 0:1], in1=s2_sb[:],
                op0=mybir.AluOpType.add, op1=mybir.AluOpType.add)
            nc.vector.tensor_scalar(
                out=negd[:], in0=d_sb[:],
                scalar1=-1.0, scalar2=None,
                op0=mybir.AluOpType.mult)

            # argmin d = first index of max(-d)  (for the idx output)
            vmax = sb.tile([BL, 8], f32)
            idx8 = sb.tile([BL, 8], mybir.dt.uint32)
            idxi = sb.tile([BL, 1], mybir.dt.int32)
            nc.vector.max(vmax[:], negd[:])
            nc.vector.max_index(idx8[:], vmax[:], negd[:])
            nc.vector.tensor_copy(out=idxi[:], in_=idx8[:, 0:1])

            # one-hot[b, p] = (-d[b,p] == max_b(-d)) — exactly one hit per
            # row (min distance gaps >> fp32 error; verified vs reference),
            # then one PE transpose to get onehot^T for the gather matmul.
            onehot_n = sb.tile([BL, P], f32)
            nc.vector.tensor_scalar(
                out=onehot_n[:], in0=negd[:],
                scalar1=vmax[:, 0:1], scalar2=None,
                op0=mybir.AluOpType.is_equal)
            ps_oh = ps.tile([P, BL], f32)
            nc.tensor.transpose(out=ps_oh[:], in_=onehot_n[:],
                                identity=ident[:])
            onehot_t = sb.tile([P, BL], f32)
            nc.vector.tensor_copy(out=onehot_t[:], in_=ps_oh[:])

            # out_seq = onehot^T.T @ proto  (exact: one 1.0 per column),
            # streamed out chunk by chunk
            NGC = 512
            for j in range((TC + NGC - 1) // NGC):
                n0 = j * NGC
                n1 = min(n0 + NGC, TC)
                ps_g = ps2.tile([BL, NGC], f32, tag="ps_g")
                nc.tensor.matmul(out=ps_g[:, 0:n1 - n0], lhsT=onehot_t[:],
                                 rhs=pr_sb[:, n0:n1], start=True, stop=True)
                g_sb = sbo.tile([BL, NGC], f32, tag="g_sb")
                if j % 2 == 0:
                    nc.scalar.copy(out=g_sb[:, 0:n1 - n0],
                                   in_=ps_g[:, 0:n1 - n0])
                else:
                    nc.vector.tensor_copy(out=g_sb[:, 0:n1 - n0],
                                          in_=ps_g[:, 0:n1 - n0])
                nc.sync.dma_start(out=oseq_d[:, n0:n1],
                                  in_=g_sb[:, 0:n1 - n0])

            nc.sync.dma_start(out=dist_d[:, :], in_=d_sb[:])
            nc.sync.dma_start(out=idx_d[:, None], in_=idxi[:])

    nc.compile()
    return nc


def _get_program():
    global _PROG
    if _PROG is None:
        _PROG = _build_program()
    return _PROG


def _pack_T(a):
    """[n, TC] row-major -> transposed, zero-padded, chunk-packed [128, KCH, n]."""
    at = np.zeros((KPAD, a.shape[0]), np.float32)
    at[:TC] = a.T
    return np.ascontiguousarray(at.reshape(KCH, 128, -1).transpose(1, 0, 2))


# set by test harness: trace the next run and stash BassKernelResults here
TRACE = False
LAST_RESULTS = None


def kernel(**inputs):
    global LAST_RESULTS
    from concourse.bass_utils import run_bass_kernel_spmd

    x = np.ascontiguousarray(np.asarray(inputs["input_seq"], dtype=np.float32))
    mask = np.ascontiguousarray(np.asarray(inputs["mask"], dtype=np.float32))
    label = np.asarray(inputs["label"])
    proto = np.ascontiguousarray(
        np.asarray(inputs["prototypes"], dtype=np.float32))

    nc = _get_program()
    pr2 = np.ascontiguousarray(proto.reshape(P, TC))
    protot_packed = _pack_T(pr2)
    ident = np.eye(BL, dtype=np.float32)

    in_maps = []
    for c in range(NCORES):
        xc = x[c * BL:(c + 1) * BL].reshape(BL, TC)
        mc = np.repeat(mask[c * BL:(c + 1) * BL], C, axis=1)
        in_maps.append({
            "xt": _pack_T(xc),
            "mt10": _pack_T(mc),
            "protot": protot_packed,
            "proto": pr2,
            "ident": ident,
        })

    res = run_bass_kernel_spmd(nc, in_maps, list(range(NCORES)), trace=TRACE)
    LAST_RESULTS = res

    outseq = np.concatenate(
        [res.results[c]["outseq"] for c in range(NCORES)], axis=0
    ).reshape(B, T, C)
    dist = np.concatenate(
        [res.results[c]["dist"] for c in range(NCORES)], axis=0)
    idx = np.concatenate(
        [res.results[c]["idx"] for c in range(NCORES)], axis=0).astype(np.int32)

    return (outseq, x, dist, idx, label, mask)
